# revision 1
# baseline (speedup 1.0000x reference)
"""Trainium2 Bass kernel: differentiable Gaussian-splat renderer.

Math: image[b,h,w,c] = clip( sum_n exp(-a_n*((gx_w-px_n)^2+(gy_h-py_n)^2)) * col[n,c], 0, 1 )
with a_n = 1/(2*sigma_n^2+1e-8), sigma_n = sizes_n*2/H.

The Gaussian separates: exp(-a*(dx^2+dy^2)) = exp(-a*dx^2)*exp(-a*dy^2), so per
frame the image is a matmul over splats:
    image[h, (w,c)] = sum_n wy[n,h] * (wx[n,w]*col[n,c])

d2 is produced by a tiny K-dim polynomial matmul on the PE:
    d2[n, g] = 1*g^2 + (-2p_n)*g + p_n^2
then ONE Exp activation with per-partition scale (-a_n) gives wx|wy.

fp32r (11-bit mantissa) would destroy the d2 cancellation for small sigma, so
both polynomial operands are split hi/lo into fp32r pairs (error-free products,
~2^-22 effective precision) -- K grows 6->18 which costs nothing on the PE.
Chunks are packed at 32-partition offsets so one PE transpose serves all three.

Sharding: data-parallel over B: 16 frames -> 8 cores x 2 frames.
"""

import numpy as np

H = 224
W = 224
NPTS = 381
CH = 3
B = 16
NCORES = 8
BPC = B // NCORES   # frames per core
NCHUNK = 3          # point chunks; n = 3*p + j  (381 = 127*3)
KC = NPTS // NCHUNK  # 127 points per chunk (contraction partitions)
NH = 336            # moving-dim half (672 = 2*336); >=256 keeps f32r at full rate

PS_TR_BUFS = 1
PS_Z_BUFS = 3
PS_OUT_BUFS = 2
T_POOL_B0 = 2  # frame-0 T-build ops with c >= this go to GPSIMD
T_POOL_B1 = 1  # frame-1 threshold
T_J2_POOL = 2  # frame-1 last chunk (tail-critical, Pool chain is the gate)
REPS = 1  # repeat whole body (benchmarking only)


def _round_f32r(x):
    """Round float32 array to fp32r (keep top 11 mantissa bits, round-nearest)."""
    u = np.ascontiguousarray(x, dtype=np.float32).view(np.uint32)
    low = u & 0xFFF
    up = (low > 0x800) | ((low == 0x800) & (((u >> 12) & 1) == 1))
    r = (u & ~np.uint32(0xFFF)) + np.where(up, np.uint32(0x1000), np.uint32(0))
    return r.view(np.float32)


def _np_consts():
    return _np_grid96()


def _np_ident():
    return np.eye(128, dtype=np.float32)


def _np_grid96():
    """[96, 448] fp32r rows; chunk j occupies rows [32j, 32j+18).
    Within a chunk, row 3r+t pairs with lhsT col t in {hi: R_hi, hi: R_lo, lo: R_hi}.
    r in 0..5 = (g^2, g, 1) for x-half cols [0:224], same for y-half [224:448]."""
    g = -1.0 + (2.0 / (W - 1)) * np.arange(W, dtype=np.float64)
    R = np.zeros((6, 2 * W), dtype=np.float64)
    R[0, 0:W] = g * g
    R[1, 0:W] = g
    R[2, 0:W] = 1.0
    R[3, W:] = g * g
    R[4, W:] = g
    R[5, W:] = 1.0
    g18 = np.zeros((18, 2 * W), dtype=np.float32)
    for r in range(6):
        hi = _round_f32r(R[r].astype(np.float32))
        lo = _round_f32r((R[r] - hi.astype(np.float64)).astype(np.float32))
        g18[3 * r + 0] = hi   # pairs L_hi
        g18[3 * r + 1] = lo   # pairs L_hi
        g18[3 * r + 2] = hi   # pairs L_lo
    out = np.zeros((96, 2 * W), dtype=np.float32)
    for j in range(NCHUNK):
        out[32 * j : 32 * j + 18] = g18
    return out


def build_bass():
    import concourse.bass as bass
    import concourse.bacc as bacc
    import concourse.tile as tile
    from concourse import mybir

    f32 = mybir.dt.float32
    f32r = mybir.dt.float32r
    Act = mybir.ActivationFunctionType
    Alu = mybir.AluOpType

    nc = bacc.Bacc("TRN2", debug=False, enable_partition_id=False)

    pk_d = nc.dram_tensor("packed", [BPC, NPTS, 6], f32, kind="ExternalInput")
    cst_d = nc.dram_tensor("consts", [96, 2 * W], f32r, kind="ExternalInput")
    id_d = nc.dram_tensor("ident", [128, 128], f32r, kind="ExternalInput")
    img_d = nc.dram_tensor("image", [BPC, H, W, CH], f32, kind="ExternalOutput")

    with tile.TileContext(nc) as tc:
        with (
            tc.tile_pool(name="const", bufs=1) as constp,
            tc.tile_pool(name="inp", bufs=1) as inp,
            tc.tile_pool(name="small", bufs=2) as small,
            tc.tile_pool(name="big", bufs=2) as big,
            tc.tile_pool(name="outp", bufs=4) as outp,
            tc.tile_pool(name="ps_tr", bufs=PS_TR_BUFS, space="PSUM") as ps_tr,
            tc.tile_pool(name="ps_z", bufs=PS_Z_BUFS, space="PSUM") as ps_z,
            tc.tile_pool(name="ps_out", bufs=PS_OUT_BUFS, space="PSUM") as ps_out,
        ):
            # ---- grid const first (gates the z matmuls), packed input second;
            # interleaved chunking: point n = 3p+j -> (partition p, chunk j)
            pk = inp.tile([128, BPC, 6 * NCHUNK], f32)  # cols (b, (j, k))
            nc.gpsimd.dma_start(
                out=pk[0:KC],
                in_=pk_d[:].rearrange("b (p j) k -> p b (j k)", j=NCHUNK),
            )
            grid96 = constp.tile([96, 2 * W], f32r)
            nc.scalar.dma_start(out=grid96, in_=cst_d[:])
            ident = constp.tile([128, 128], f32r)
            nc.sync.dma_start(out=ident, in_=id_d[:])
            # f32 zero source for initializing f32r pad columns (f32r memset
            # is invalid ISA; a rounding tensor_copy is the legal producer)
            zpad = constp.tile([128, 14 * NCHUNK], f32)
            nc.vector.memset(zpad, 0.0)

            for _rep in range(REPS):
                # ---- PE warmup: dummy matmuls so the HAM clock-gate opens
                # before the real pipeline reaches the PE
                wsb = constp.tile([1, 128], f32)
                nc.vector.memset(wsb, 0.0)
                wps = ps_tr.tile([128, 128], f32, tag="tr")
                for _ in range(4):
                    nc.tensor.matmul(wps, wsb, wsb, start=True, stop=True)

                # ---- both frames' param chains batched into single wide ops
                # an = -1/(2*sigma^2 + 1e-8)  [*, b, j]
                szr = pk.rearrange("p b (j k) -> p b j k", k=6)[:, :, :, 5]
                sq = small.tile([128, BPC, NCHUNK], f32, tag="sq")
                nc.scalar.activation(
                    out=sq[0:KC], in_=szr[0:KC], func=Act.Square, scale=2.0 / H
                )
                u = small.tile([128, BPC, NCHUNK], f32, tag="u")
                nc.vector.tensor_scalar(
                    out=u[0:KC], in0=sq[0:KC], scalar1=-2.0, scalar2=-1e-8,
                    op0=Alu.mult, op1=Alu.add,
                )
                an2 = small.tile([128, BPC, NCHUNK], f32, tag="an")
                nc.vector.reciprocal(out=an2[0:KC], in_=u[0:KC])

                # ---- polynomial coeffs L6[p, b, 6j+r] = (1,-2px,px^2, 1,-2py,py^2)
                pkr = pk.rearrange("p b (j k) -> p b j k", k=6)
                posr = pkr[:, :, :, 0:2]   # [*, b, j, e]
                L6 = small.tile([128, BPC, 6 * NCHUNK], f32, tag="L6")
                nc.vector.memset(L6, 0.0)
                axr = L6.rearrange("p b (j s r) -> p b j s r", s=2, r=3)[0:KC]
                nc.vector.memset(axr[:, :, :, :, 0], 1.0)
                nc.vector.tensor_scalar(
                    out=axr[:, :, :, :, 1], in0=posr[0:KC], scalar1=-2.0,
                    scalar2=None, op0=Alu.mult,
                )
                nc.vector.tensor_mul(axr[:, :, :, :, 2], posr[0:KC], posr[0:KC])

                # ---- fp32r hi/lo split, chunks packed at 32-col offsets:
                # Lsplit[p, b, 32j+3r+t], t = (hi, hi, lo); cols 18..31 of each
                # block pair with zero grid rows so their content is irrelevant.
                Lsplit2 = small.tile([128, BPC, 96], f32r, tag="Lsp")
                Lspr2 = Lsplit2.rearrange("p b (j c) -> p b j c", j=NCHUNK)
                for bb in range(BPC):
                    nc.vector.tensor_copy(
                        out=Lspr2[:, bb, :, 18:32],
                        in_=zpad.rearrange("p (j c) -> p j c", j=NCHUNK),
                    )
                Lsp3 = Lspr2[
                    :, :, :, 0:18
                ].rearrange("p b j (r t) -> p b j r t", t=3)
                L6j = L6.rearrange("p b (j r) -> p b j r", r=6)
                nc.vector.tensor_copy(out=Lsp3[:, :, :, :, 0], in_=L6j)
                nc.vector.tensor_copy(out=Lsp3[:, :, :, :, 1], in_=L6j)
                lo = small.tile([128, BPC, 6 * NCHUNK], f32, tag="lo")
                lor = lo.rearrange("p b (j r) -> p b j r", r=6)
                nc.vector.tensor_tensor(
                    out=lor, in0=L6j, in1=Lsp3[:, :, :, :, 0].bitcast(f32),
                    op=Alu.subtract,
                )
                nc.vector.tensor_copy(out=Lsp3[:, :, :, :, 2], in_=lor)

                # ---- pass 1 (both frames): transpose, d2 matmuls, exp, T —
                # keeps the cheap PE ops ahead of the long main-MM stream so
                # the activation engine is never starved
                wxy_b, T_b = [], []
                for b in range(BPC):
                    colr = pkr[:, b][:, :, 2:5]   # [*, j, c]
                    an = an2[:, b]

                    # one transpose for all chunks -> lhs96 [96, 128]
                    tr = ps_tr.tile([96, 128], f32r, tag="tr")
                    nc.tensor.transpose(tr, Lsplit2[:, b], ident)
                    lhs96 = small.tile([96, 128], f32r, tag="lhs96")
                    nc.vector.tensor_copy(out=lhs96, in_=tr)

                    # per chunk: d2 matmul (K=18 at partition 32j); exp
                    wxy_all = big.tile([128, NCHUNK, 2 * W], f32r, tag="wxy")
                    T_all = big.tile([128, NCHUNK, CH * W], f32r, tag="T")
                    Twc = T_all.rearrange("p j (w c) -> p j w c", c=CH)
                    for j in range(NCHUNK):
                        z = ps_z.tile([128, 2 * W], f32, tag="z")
                        nc.tensor.matmul(
                            z,
                            lhs96[32 * j : 32 * j + 18, :],
                            grid96[32 * j : 32 * j + 18, :],
                            start=True, stop=True,
                        )
                        nc.scalar.activation(
                            out=wxy_all[0:KC, j, :], in_=z[0:KC, :], func=Act.Exp,
                            scale=an[0:KC, j : j + 1],
                        )
                        # T[p, j, 3w+c] = wx[p,j,w] * col[p,(j,c)]  (w-major)
                        for c in range(CH):
                            tpc = T_POOL_B0 if b == 0 else T_POOL_B1
                            if b == 1 and j == NCHUNK - 1:
                                tpc = T_J2_POOL
                            eng = nc.gpsimd if c >= tpc else nc.vector
                            eng.tensor_scalar(
                                out=Twc[0:KC, j, :, c],
                                in0=wxy_all[0:KC, j, 0:W].bitcast(f32),
                                scalar1=colr[0:KC, j, c : c + 1],
                                scalar2=None,
                                op0=Alu.mult,
                            )
                    wxy_b.append(wxy_all)
                    T_b.append(T_all)

                # ---- pass 2 (both frames): main matmuls into per-h-chunk
                # 2-bank psum tiles; bank x = half holds 3w+c in [336x, 336x+336)
                for b in range(BPC):
                    wxy_all, T_all = wxy_b[b], T_b[b]
                    for i, (h0, hsz) in enumerate(((0, 128), (128, 96))):
                        po = ps_out.tile([128, 2, 512], f32, tag="out")
                        for j in range(NCHUNK):
                            for half in range(2):
                                nc.tensor.matmul(
                                    po[0:hsz, half, 0:NH],
                                    wxy_all[0:KC, j, W + h0 : W + h0 + hsz],
                                    T_all[0:KC, j, NH * half : NH * (half + 1)],
                                    start=(j == 0), stop=(j == NCHUNK - 1),
                                )
                        osb = outp.tile([128, W * CH], f32, tag="osb")
                        nc.vector.tensor_scalar(
                            out=osb[0:hsz].rearrange("p (x s) -> p x s", s=NH),
                            in0=po[0:hsz, :, 0:NH],
                            scalar1=1.0, scalar2=None, op0=Alu.min,
                        )
                        # alternate queues so consecutive transfers overlap
                        dma_eng = nc.scalar if (2 * b + i + 1) % 2 else nc.sync
                        dma_eng.dma_start(
                            out=img_d[b, h0 : h0 + hsz].rearrange(
                                "h w c -> h (w c)"
                            ),
                            in_=osb[0:hsz],
                        )
    nc.compile()
    return nc


_CACHED = {}


def _get_bass():
    if "nc" not in _CACHED:
        _CACHED["nc"] = build_bass()
    return _CACHED["nc"]


def _pack_inputs(positions, colors, sizes):
    pk = np.empty((positions.shape[0], NPTS, 6), dtype=np.float32)
    pk[:, :, 0:2] = positions
    pk[:, :, 2:5] = colors
    pk[:, :, 5] = sizes
    return pk


LAST_RESULT = None


def kernel(positions, colors, sizes, trace=False):
    from concourse.bass_utils import run_bass_kernel_spmd

    global LAST_RESULT
    positions = np.ascontiguousarray(np.asarray(positions, dtype=np.float32))
    colors = np.ascontiguousarray(np.asarray(colors, dtype=np.float32))
    sizes = np.ascontiguousarray(np.asarray(sizes, dtype=np.float32))

    pk = _pack_inputs(positions, colors, sizes)
    cst = _np_consts()
    nc = _get_bass()
    in_maps = []
    for c in range(NCORES):
        sl = slice(c * BPC, (c + 1) * BPC)
        in_maps.append(
            {"packed": pk[sl], "consts": cst, "ident": _np_ident(),
}
        )

    res = run_bass_kernel_spmd(
        nc, in_maps, core_ids=list(range(NCORES)), trace=trace
    )
    LAST_RESULT = res
    return np.concatenate([r["image"] for r in res.results], axis=0)


def _exec_fn(nc):
    """Build a reusable jitted 8-core executor (no donation; kernel writes
    every output element so uninit result buffers are fine)."""
    import jax
    from jax.experimental.shard_map import shard_map
    from jax.sharding import Mesh, PartitionSpec
    from concourse import bass2jax, mybir

    bass2jax.install_neuronx_cc_hook()

    in_names, out_names, out_avals = [], [], []
    for alloc in nc.m.functions[0].allocations:
        if not isinstance(alloc, mybir.MemoryLocationSet):
            continue
        name = alloc.memorylocations[0].name
        if alloc.kind == "ExternalInput":
            in_names.append(name)
        elif alloc.kind == "ExternalOutput":
            out_names.append(name)
            out_avals.append(
                jax.core.ShapedArray(
                    tuple(alloc.tensor_shape), mybir.dt.np(alloc.dtype)
                )
            )
    all_in = in_names + out_names

    def _body(*args):
        outs = bass2jax._bass_exec_p.bind(
            *args,
            out_avals=tuple(out_avals),
            in_names=tuple(all_in),
            out_names=tuple(out_names),
            lowering_input_output_aliases=(),
            sim_require_finite=True,
            sim_require_nnan=True,
            nc=nc,
        )
        return tuple(outs)

    devices = jax.devices()[:NCORES]
    mesh = Mesh(np.asarray(devices), ("core",))
    n_args = len(all_in)
    sharded = jax.jit(
        shard_map(
            _body,
            mesh=mesh,
            in_specs=(PartitionSpec("core"),) * n_args,
            out_specs=(PartitionSpec("core"),) * len(out_names),
            check_rep=False,
        ),
        keep_unused=True,
    )
    return sharded, mesh, in_names, out_names, out_avals


def bench(positions, colors, sizes, iters=50):
    """Steady-state per-execution wall time (s) over 8 cores + output."""
    import time as _time
    import jax
    from jax.sharding import NamedSharding, PartitionSpec

    positions = np.ascontiguousarray(np.asarray(positions, dtype=np.float32))
    colors = np.ascontiguousarray(np.asarray(colors, dtype=np.float32))
    sizes = np.ascontiguousarray(np.asarray(sizes, dtype=np.float32))
    nc = _get_bass()
    sharded, mesh, in_names, out_names, out_avals = _exec_fn(nc)

    feed = {
        "packed": _pack_inputs(positions, colors, sizes),
        "consts": np.concatenate([_np_consts()] * NCORES, axis=0),
        "ident": np.concatenate([_np_ident()] * NCORES, axis=0),
    }
    args = [feed[n] for n in in_names]
    args += [
        np.zeros((NCORES * a.shape[0], *a.shape[1:]), a.dtype) for a in out_avals
    ]
    sh = NamedSharding(mesh, PartitionSpec("core"))
    dargs = [jax.device_put(a, sh) for a in args]

    out = sharded(*dargs)
    jax.block_until_ready(out)
    img0 = np.asarray(out[0]).reshape(NCORES, BPC, H, W, CH).reshape(B, H, W, CH)

    times = []
    for _ in range(3):
        t0 = _time.perf_counter()
        for _ in range(iters):
            out = sharded(*dargs)
        jax.block_until_ready(out)
        times.append((_time.perf_counter() - t0) / iters)
    return min(times), img0

